# revision 18
# baseline (speedup 1.0000x reference)
"""Trainium2 Bass kernel for DecoderWithTemporalAttention (single-step decode).

Math (reference collapses to, since initial decoder state is zero):
    re1    = tanh(h @ Ud_w.T + (Ud_b + Wd_b))          # [B, T, E]
    scores = re1 @ vd_w[0]                              # [B, T]  (+vd_b, dropped: softmax-invariant)
    beta   = softmax(scores, axis=T)
    c_t    = einsum('bt,bte->be', beta, h)              # [B, E]
    y_til  = concat([c_t, y[:,0]], -1) @ wt_w[0] + wt_b # [B]
    gates  = outer(y_til, W_ih[:,0]) + (b_ih + b_hh)    # [B, 4H]
    i,f,g,o = split(gates); d_new = sigmoid(o) * tanh(sigmoid(i) * tanh(g))
    returns stack([d_new, c_t])                         # [2, B, 256]

Sharding: pure data-parallel, batch 256 -> 8 cores x 32.

Device layout per core (BL=32 local batches, group = 4 batches = 2 pairs):
  - one 1MB bf16 DMA per pair: [h natural | hT] packed per partition
  - single packed constants blob -> one DMA covers all main-loop constants
  - ud matmul: stationary Ud_w.T bf16, out [e_out(128x2), (r, t)] PSUM
  - tanh+bias on ScalarE (per-partition bias) -> re1 bf16 in SBUF
  - scores: M=1 matmuls vd-stationary, 4 batches packed per PSUM bank at
    partitions {0,32,64,96} via col-tiling
  - softmax directly on the PSUM scores (max / exp+accum_out / recip);
    pexp (unnormalized) kept bf16, normalization deferred to c_t evacuation
  - pexp rows gathered + PE-transposed -> pexpT columns [t, 4]
  - c_t: M=1 matmuls, stationary pexpT column, moving natural h -> PSUM,
    evacuated with a fused 1/Z tensor_scalar_mul
  - y_tilde partial: fused tensor_tensor_reduce, per-group gather into yt_row
  - LSTM tail: bf16 K=1 outer-product gates matmul; sigmoid via
    0.5*(1+tanh(x/2)) so the whole kernel uses one ACT table set
"""

import os
import sys

import numpy as np

for _p in ("/opt/trn_rl_repo",):
    if _p not in sys.path and os.path.isdir(_p):
        sys.path.append(_p)

B, T, HE, HD = 256, 512, 256, 256
NCORES = 8
BL = B // NCORES  # 32 local batches per core
G4 = BL // 4      # groups of 4 batches

_cache = {}
DEBUG_GROUPS = int(os.environ.get("KERN_GROUPS", str(G4)))
DEBUG_STAGE = int(os.environ.get("KERN_STAGE", "99"))
SKIP_TAIL = os.environ.get("KERN_SKIP_TAIL", "0") == "1"
HBF = 2048  # bf16 cols of one batch-pair's natural-h region
TBF = 2048  # bf16 cols of one batch-pair's transposed region
PF = HBF + TBF  # bf16 cols per pair

# constants blob layout (f32 cols)
CB_UDT0 = 0       # [128,256] bf16 -> 128 f32 cols
CB_UDT1 = 128
CB_VD2 = 256      # [128,2] bf16 -> 1
CB_IDENT = 257    # [128,128] bf16 -> 64
CB_WDB2 = 321     # [128,2] f32
CB_WTR = 323      # [128,256] f32
CB_C2 = 579       # [128,G4] f32
CB_COLS = 579 + G4


def _build_nc():
    from concourse import bacc, bass, mybir
    from concourse.tile import TileContext

    f32 = mybir.dt.float32
    bf16 = mybir.dt.bfloat16
    AF = mybir.ActivationFunctionType
    ALU = mybir.AluOpType

    nc = bacc.Bacc()

    # one blob per group of 4 batches: [group, partition, pair(2) x (h | hT)]
    hh_d = nc.declare_dram_parameter("hh", [G4, 128, 2 * PF], bf16, isOutput=False)
    cb_d = nc.declare_dram_parameter("cb", [128, CB_COLS], f32, isOutput=False)
    wih_d = nc.declare_dram_parameter("wih", [1, 4 * HD], bf16, isOutput=False)
    bsum_d = nc.declare_dram_parameter("bsum", [BL, 4 * HD], f32, isOutput=False)
    outd_d = nc.declare_dram_parameter("out_d", [BL, HD], f32, isOutput=True)
    outc_d = nc.declare_dram_parameter("out_c", [BL, HE], f32, isOutput=True)

    with TileContext(nc) as tc:
        with (
            tc.tile_pool(name="const", bufs=1) as constp,
            tc.tile_pool(name="hp", bufs=6) as hp,
            tc.tile_pool(name="re1p", bufs=3) as re1p,
            tc.tile_pool(name="smp", bufs=3) as smp,
            # tiles read by a DMA (gather/out): never recycle their slots, so
            # no engine op ever carries a WAR wait on a DMA semaphore
            tc.tile_pool(name="nrp", bufs=G4) as nrp,
            tc.tile_pool(name="tailp", bufs=1) as tailp,
            tc.tile_pool(name="udps", bufs=2, space="PSUM") as udps,
            tc.tile_pool(name="scps", bufs=1, space="PSUM") as scps,
            tc.tile_pool(name="trps", bufs=1, space="PSUM") as trps,
            tc.tile_pool(name="ctps", bufs=2, space="PSUM") as ctps,
        ):
            # ---- first h loads go on the ring ahead of everything ----
            prefetch = []
            for q in range(3):
                g0, qq0 = divmod(q, 2)
                t_ = hp.tile([128, PF], bf16, tag="h")
                nc.sync.dma_start(
                    out=t_[:], in_=hh_d[g0][:, qq0 * PF : (qq0 + 1) * PF]
                )
                prefetch.append(t_)

            # ---- constants: one packed DMA + two tail-only DMAs ----
            cb = constp.tile([128, CB_COLS], f32, tag="cb")
            nc.sync.dma_start(out=cb[:], in_=cb_d[:])
            udt_sb = [
                cb[:, CB_UDT0 : CB_UDT0 + 128].bitcast(bf16),
                cb[:, CB_UDT1 : CB_UDT1 + 128].bitcast(bf16),
            ]
            vd2_sb = cb[:, CB_VD2 : CB_VD2 + 1].bitcast(bf16)
            ident_sb = cb[:, CB_IDENT : CB_IDENT + 64].bitcast(bf16)
            wdb2_sb = cb[:, CB_WDB2 : CB_WDB2 + 2]
            wtr_sb = cb[:, CB_WTR : CB_WTR + HE]
            c2_sb = cb[:, CB_C2 : CB_C2 + G4]
            wih_sb = constp.tile([1, 4 * HD], bf16, tag="wih")
            nc.sync.dma_start(out=wih_sb[:], in_=wih_d[:])
            bsum_sb = constp.tile([BL, 4 * HD], f32, tag="bsum")
            nc.sync.dma_start(out=bsum_sb[:], in_=bsum_d[:])

            # ---- per-engine warmups ----
            # Every engine observes each constant-DMA semaphore via a cheap op
            # up front; later ops then need <=1 sync wait (the hardware
            # instruction structs encode only one wait command).
            warm = ctps.tile([128, T], f32, tag="ct")
            nc.tensor.matmul(
                warm[0:1, 0:1], vd2_sb[0:1, 0:1], vd2_sb[0:1, 0:1],
                start=True, stop=True,
            )
            nc.tensor.matmul(
                warm[0:1, 1:2], wih_sb[0:1, 0:1], wih_sb[0:1, 0:1],
                start=True, stop=True,
            )
            dscr = tailp.tile([1, 8], f32, tag="dscr")
            nc.vector.tensor_copy(dscr[0:1, 0:1], wtr_sb[0:1, 0:1])
            nc.vector.tensor_copy(dscr[0:1, 1:2], bsum_sb[0:1, 0:1])
            ascr = tailp.tile([1, 8], f32, tag="ascr")
            nc.scalar.copy(ascr[0:1, 0:1], wdb2_sb[0:1, 0:1])
            # chained: materialize the float-bias const AP on ACT, and force
            # the (single) tanh/exp table-set load before the main loop
            nc.scalar.activation(ascr[0:1, 1:2], ascr[0:1, 0:1], AF.Tanh, bias=0.0)

            ytacc = tailp.tile([128, G4], f32, tag="ytacc")

            def absorb(ps):
                # tiny const matmul into a freshly allocated PSUM tile: takes
                # over the slot-release wait so the first real matmul into the
                # tile carries only its own (single) cross-engine wait
                nc.tensor.matmul(
                    ps[0:1, 0:1], vd2_sb[0:1, 0:1], vd2_sb[0:1, 0:1],
                    start=True, stop=True,
                )

            # ---- per-group-of-4 pipeline ----
            for g in range(DEBUG_GROUPS):
                sc = scps.tile([128, T], f32, tag="sc")
                absorb(sc)
                h_pair = [None, None]
                for qq in range(2):  # two pairs of batches in this group
                    if 2 * g + qq < len(prefetch):
                        hh_sb = prefetch[2 * g + qq]
                    else:
                        hh_sb = hp.tile([128, PF], bf16, tag="h")
                        nc.sync.dma_start(
                            out=hh_sb[:], in_=hh_d[g][:, qq * PF : (qq + 1) * PF]
                        )
                    h_pair[qq] = hh_sb[:, 0:HBF]
                    hT_sb = hh_sb[:, HBF:PF]
                    # ud matmuls for both batches of the pair; ei-outer /
                    # r-inner keeps each stationary loaded for 2 matmuls
                    ud_eo = []
                    for eo in range(2):
                        ud = udps.tile([128, 2 * T], f32, tag="ud")
                        absorb(ud)
                        ud_eo.append(ud)
                        for ei in range(2):
                            for r in range(2):
                                nc.tensor.matmul(
                                    ud[:, r * T : (r + 1) * T],
                                    udt_sb[ei][:, eo * 128 : (eo + 1) * 128],
                                    hT_sb[:, (r * 2 + ei) * T : (r * 2 + ei + 1) * T],
                                    start=(ei == 0),
                                    stop=(ei == 1),
                                )
                    # tanh(+bias) over the pair -> re1 bf16 [128, (eo, r, t)]
                    re1 = re1p.tile([128, 2 * 2 * T], bf16, tag="re1")
                    for eo in range(2):
                        nc.scalar.activation(
                            re1[:, eo * 2 * T : (eo + 1) * 2 * T],
                            ud_eo[eo][:],
                            AF.Tanh,
                            bias=wdb2_sb[:, eo : eo + 1],
                        )
                    if DEBUG_STAGE <= 1:
                        continue
                    # scores for both batches -> psum rows {0,32,64,96}
                    # (r outer: each strip's accumulation group opens and
                    # closes before the next strip's begins — one pending
                    # group per PSUM bank)
                    for r in range(2):
                        for ei in range(2):
                            jj = 2 * qq + r
                            nc.tensor.matmul(
                                sc[32 * jj : 32 * jj + 32, :],
                                vd2_sb[:, ei : ei + 1].broadcast_to([128, 32]),
                                re1[:, (ei * 2 + r) * T : (ei * 2 + r + 1) * T],
                                start=(ei == 0),
                                stop=(ei == 1),
                                tile_position=(0, 32 * jj),
                            )

                if DEBUG_STAGE <= 1:
                    continue
                # ---- softmax over this group of 4, directly on PSUM ----
                # scores are bounded (|s| <= sum|vd| ~ 10) so no max-shift is
                # needed; score rows are replicated across each 32-partition
                # strip, so every partition's zsum is its strip's batch value.
                pexp = nrp.tile([128, T], bf16, tag="pexp")
                zsum = smp.tile([128, 1], f32, tag="zsum")
                nc.scalar.activation(pexp[:], sc[:], AF.Exp, accum_out=zsum[:])
                rz = smp.tile([128, 1], f32, tag="rz")
                nc.vector.reciprocal(rz[:], zsum[:])
                # gather the 4 pexp rows into contiguous partitions (DMA moves
                # freely across partitions; engines cannot); SWDGE keeps this
                # latency-bound transfer off the bulk h ring and the busy ACT
                beta = smp.tile([4, T], bf16, tag="beta")
                # DVE dummy write absorbs the slot-release wait so the gather
                # DMA carries only its producer wait
                nc.vector.tensor_copy(beta[0:1, 0:1], dscr[0:1, 0:1])
                nc.gpsimd.dma_start(
                    out=beta[:],
                    in_=pexp[:].rearrange("(j s) t -> j s t", s=32)[:, 0, :],
                )

                # ---- transpose pexp -> pexpT columns [t=128 x 4tt, batch] ----
                btr = trps.tile([128, 32], bf16, tag="btr")
                absorb(btr[:, 0:2].bitcast(f32))
                for tt in range(4):
                    nc.tensor.transpose(
                        btr[:, tt * 4 : (tt + 1) * 4],
                        beta[:, tt * 128 : (tt + 1) * 128],
                        ident_sb[0:4, 0:4],
                    )
                betaT = smp.tile([128, 16], bf16, tag="betaT")
                nc.vector.tensor_copy(betaT[:], btr[:, 0:16])

                if DEBUG_STAGE <= 4:
                    nc.sync.dma_start(
                        out=outc_d[4 * g : 4 * g + 4, 0:16], in_=betaT[0:4, 0:16]
                    )
                    continue
                # ---- c_t for the 4 batches (unnormalized, then fused 1/Z) ----
                ct = ctps.tile([128, T], f32, tag="ct")
                absorb(ct)
                for jj in range(4):
                    qq, r = divmod(jj, 2)
                    for tt in range(4):
                        nc.tensor.matmul(
                            ct[32 * jj : 32 * jj + 32, 0:HE],
                            betaT[:, tt * 4 + jj : tt * 4 + jj + 1].broadcast_to(
                                [128, 32]
                            ),
                            h_pair[qq][:, (r * 4 + tt) * HE : (r * 4 + tt + 1) * HE],
                            start=(tt == 0),
                            stop=(tt == 3),
                            tile_position=(0, 32 * jj),
                        )
                ctstage = nrp.tile([128, HE], f32, tag="ctstage")
                nc.vector.tensor_scalar_mul(ctstage[:], ct[:, 0:HE], rz[:])
                # c_t output rows for this group (DMA un-strides the rows)
                nc.sync.dma_start(
                    out=outc_d[4 * g : 4 * g + 4, :],
                    in_=ctstage[:].rearrange("(j s) e -> j s e", s=32)[:, 0, :],
                )
                if DEBUG_STAGE <= 5:
                    continue
                # y_tilde partial for this group: wt.c_t dot into this
                # group's ytacc column (tensor_tensor_reduce would fuse these
                # two but crashes the hardware — sim-only support)
                wscr = smp.tile([128, HE], f32, tag="wscr")
                nc.vector.tensor_mul(wscr[:], ctstage[:], wtr_sb[:])
                nc.vector.reduce_sum(
                    ytacc[:, g : g + 1], wscr[:], axis=mybir.AxisListType.X
                )

            # ---- tail: gates, LSTM cell (sigmoid via 0.5*(1+tanh(x/2))) ----
            if SKIP_TAIL or DEBUG_STAGE <= 5:
                pass
            else:
                yts = nrp.tile([128, G4], bf16, tag="yts")
                nc.vector.tensor_add(yts[:], ytacc[:], c2_sb[:])
                # gather strided y_tilde entries into one [1, BL] stationary
                # row; position p = j*8+g holds batch 4g+j (the scramble is
                # undone by the out_d DMA access pattern below)
                yt_row = tailp.tile([1, BL], bf16, tag="yt_row")
                nc.gpsimd.dma_start(
                    out=yt_row[:],
                    in_=yts[:].rearrange("(j s) g -> j s g", s=32)[:, 0, :],
                )
                gsb = tailp.tile([BL, 4 * HD], f32, tag="gsb")
                for half in range(2):
                    gps = ctps.tile([128, T], f32, tag="ct")
                    absorb(gps)
                    nc.tensor.matmul(
                        gps[0:BL, 0:512],
                        yt_row[:],
                        wih_sb[:, half * 512 : (half + 1) * 512],
                        start=True,
                        stop=True,
                    )
                    nc.vector.tensor_add(
                        gsb[:, half * 512 : (half + 1) * 512],
                        gps[0:BL, 0:512],
                        bsum_sb[:, half * 512 : (half + 1) * 512],
                    )
                # gates: i=[0:256], g=[512:768], o=[768:1024]  (f unused: c0=0)
                ti = tailp.tile([BL, HD], f32, tag="ti")
                nc.scalar.activation(ti[:], gsb[:, 0:256], AF.Tanh, scale=0.5)
                tg = tailp.tile([BL, HD], f32, tag="tg")
                nc.scalar.activation(tg[:], gsb[:, 512:768], AF.Tanh)
                to = tailp.tile([BL, HD], f32, tag="to")
                nc.scalar.activation(to[:], gsb[:, 768:1024], AF.Tanh, scale=0.5)
                # cn2 = (ti+1)*tg = 2*c_new ; tanh(0.5*cn2) = tanh(c_new)
                cn2 = tailp.tile([BL, HD], f32, tag="cn2")
                nc.vector.scalar_tensor_tensor(
                    out=cn2[:], in0=ti[:], scalar=1.0, in1=tg[:],
                    op0=ALU.add, op1=ALU.mult,
                )
                tcn = tailp.tile([BL, HD], f32, tag="tcn")
                nc.scalar.activation(tcn[:], cn2[:], AF.Tanh, scale=0.5)
                # dn2 = (to+1)*tcn = 2*d_new
                dn2 = tailp.tile([BL, HD], f32, tag="dn2")
                nc.vector.scalar_tensor_tensor(
                    out=dn2[:], in0=to[:], scalar=1.0, in1=tcn[:],
                    op0=ALU.add, op1=ALU.mult,
                )
                dnew = tailp.tile([BL, HD], f32, tag="dnew")
                nc.vector.tensor_scalar_mul(dnew[:], dn2[:], 0.5)

                # dnew row j*8+g is batch 4g+j
                nc.sync.dma_start(
                    out=outd_d[:].rearrange("(g j) e -> j g e", j=4), in_=dnew[:]
                )

    nc.compile()
    return nc


def _prep_in_maps(inputs):
    h = np.asarray(inputs["h_t_enc"], np.float32)
    y = np.asarray(inputs["y"], np.float32)
    Ud_w = np.asarray(inputs["Ud_w"], np.float32)
    Ud_b = np.asarray(inputs["Ud_b"], np.float32)
    Wd_b = np.asarray(inputs["Wd_b"], np.float32)
    vd_w = np.asarray(inputs["vd_w"], np.float32)
    wt_w = np.asarray(inputs["wt_w"], np.float32)
    wt_b = np.asarray(inputs["wt_b"], np.float32)
    W_ih = np.asarray(inputs["W_ih"], np.float32)
    b_ih = np.asarray(inputs["b_ih"], np.float32)
    b_hh = np.asarray(inputs["b_hh"], np.float32)

    from ml_dtypes import bfloat16

    udt = np.ascontiguousarray(Ud_w.T).astype(bfloat16)  # [256,256]
    vd2 = np.ascontiguousarray(vd_w[0].reshape(2, 128).T).astype(bfloat16)
    wdb2 = np.ascontiguousarray((Wd_b + Ud_b).reshape(2, 128).T)
    wtr = np.tile(wt_w[0][:HE][None, :], (128, 1)).astype(np.float32)
    wih = W_ih[:, 0][None, :].astype(bfloat16)
    bsum = np.tile((b_ih + b_hh)[None, :], (BL, 1)).astype(np.float32)
    ident = np.eye(128, dtype=bfloat16)

    def make_hh(hc):
        # h-region: per pair q, partition p: [rb(2), tt(4), e(256)] natural rows
        hp_ = np.ascontiguousarray(
            hc.reshape(BL // 2, 2, 4, 128, HE).transpose(0, 3, 1, 2, 4)
            .reshape(BL // 2, 128, 2048)
        ).astype(bfloat16)
        # hT-region: [rb(2), et(2), t(512)] transposed rows
        ht_ = np.ascontiguousarray(
            hc.transpose(0, 2, 1).reshape(BL // 2, 2, 2, 128, T)
            .transpose(0, 3, 1, 2, 4).reshape(BL // 2, 128, 2048)
        ).astype(bfloat16)
        pair = np.concatenate([hp_, ht_], axis=2)  # [BL//2, 128, PF]
        # group blob: both pairs of a group side by side per partition
        return np.ascontiguousarray(
            pair.reshape(G4, 2, 128, PF).transpose(0, 2, 1, 3)
            .reshape(G4, 128, 2 * PF)
        )

    def f32view(a):
        return np.ascontiguousarray(a).view(np.float32)

    in_maps = []
    for c in range(NCORES):
        sl = slice(c * BL, (c + 1) * BL)
        hc = h[sl]
        # per-batch constant part of y_tilde at strided layout [32*jj, g]
        c2v = wt_w[0, HE] * y[sl, 0, 0] + wt_b[0]  # [BL]
        c2 = np.zeros((128, G4), np.float32)
        for b in range(BL):
            c2[32 * (b % 4), b // 4] = c2v[b]
        cbl = np.concatenate(
            [
                f32view(udt[0:128]),
                f32view(udt[128:256]),
                f32view(vd2),
                f32view(ident),
                wdb2,
                wtr,
                c2,
            ],
            axis=1,
        )
        assert cbl.shape == (128, CB_COLS), cbl.shape
        in_maps.append(
            {
                "hh": make_hh(hc),
                "cb": np.ascontiguousarray(cbl),
                "wih": wih,
                "bsum": bsum,
            }
        )
    return in_maps


def kernel(**inputs):
    from concourse.bass_utils import run_bass_kernel_spmd

    key = 0
    if key not in _cache:
        _cache[key] = _build_nc()
    nc = _cache[key]

    in_maps = _prep_in_maps(inputs)
    res = run_bass_kernel_spmd(nc, in_maps, list(range(NCORES)))
    kernel.last_results = res

    d_new = np.concatenate([np.asarray(r["out_d"]) for r in res.results], axis=0)
    c_t = np.concatenate([np.asarray(r["out_c"]) for r in res.results], axis=0)
    return np.stack([d_new.astype(np.float32), c_t.astype(np.float32)], axis=0)


kernel.last_results = None


# revision 22
# speedup vs baseline: 1.1217x; 1.1217x over previous
"""Trainium2 Bass kernel for DecoderWithTemporalAttention (single-step decode).

Math (reference collapses to, since initial decoder state is zero):
    re1    = tanh(h @ Ud_w.T + (Ud_b + Wd_b))          # [B, T, E]
    scores = re1 @ vd_w[0]                              # [B, T]  (+vd_b, dropped: softmax-invariant)
    beta   = softmax(scores, axis=T)
    c_t    = einsum('bt,bte->be', beta, h)              # [B, E]
    y_til  = concat([c_t, y[:,0]], -1) @ wt_w[0] + wt_b # [B]
    gates  = outer(y_til, W_ih[:,0]) + (b_ih + b_hh)    # [B, 4H]
    i,f,g,o = split(gates); d_new = sigmoid(o) * tanh(sigmoid(i) * tanh(g))
    returns stack([d_new, c_t])                         # [2, B, 256]

Sharding: pure data-parallel, batch 256 -> 8 cores x 32.

Device layout per core (BL=32 local batches, group = 4 batches = 2 pairs):
  - one 1MB bf16 DMA per pair: [h natural | hT] packed per partition
  - single packed constants blob -> one DMA covers all main-loop constants
  - ud matmul: stationary Ud_w.T bf16, out [e_out(128x2), (r, t)] PSUM
  - tanh+bias on ScalarE (per-partition bias) -> re1 bf16 in SBUF
  - scores: M=1 matmuls vd-stationary, 4 batches packed per PSUM bank at
    partitions {0,32,64,96} via col-tiling
  - softmax directly on the PSUM scores (max / exp+accum_out / recip);
    pexp (unnormalized) kept bf16, normalization deferred to c_t evacuation
  - pexp rows gathered + PE-transposed -> pexpT columns [t, 4]
  - c_t: M=1 matmuls, stationary pexpT column, moving natural h -> PSUM,
    evacuated with a fused 1/Z tensor_scalar_mul
  - y_tilde partial: fused tensor_tensor_reduce, per-group gather into yt_row
  - LSTM tail: bf16 K=1 outer-product gates matmul; sigmoid via
    0.5*(1+tanh(x/2)) so the whole kernel uses one ACT table set
"""

import os
import sys

import numpy as np

for _p in ("/opt/trn_rl_repo",):
    if _p not in sys.path and os.path.isdir(_p):
        sys.path.append(_p)

B, T, HE, HD = 256, 512, 256, 256
NCORES = 8
BL = B // NCORES  # 32 local batches per core
G4 = BL // 4      # groups of 4 batches

_cache = {}
DEBUG_GROUPS = int(os.environ.get("KERN_GROUPS", str(G4)))
DEBUG_STAGE = int(os.environ.get("KERN_STAGE", "99"))
SKIP_TAIL = os.environ.get("KERN_SKIP_TAIL", "0") == "1"
HBF = 2048  # bf16 cols of one batch-pair's natural-h region
TBF = 2048  # bf16 cols of one batch-pair's transposed region
PF = HBF + TBF  # bf16 cols per pair

# constants blob layout (f32 cols)
CB_UDT0 = 0       # [128,256] bf16 -> 128 f32 cols
CB_UDT1 = 128
CB_VD2 = 256      # [128,2] bf16 -> 1
CB_IDENT = 257    # [128,128] bf16 -> 64
CB_WDB2 = 321     # [128,2] f32
CB_WTR = 323      # [128,256] f32
CB_C2 = 579       # [128,G4] f32
CB_COLS = 579 + G4


def _build_nc():
    from concourse import bacc, bass, mybir
    from concourse.tile import TileContext

    f32 = mybir.dt.float32
    bf16 = mybir.dt.bfloat16
    AF = mybir.ActivationFunctionType
    ALU = mybir.AluOpType

    nc = bacc.Bacc()

    # one blob per group of 4 batches: [group, partition, pair(2) x (h | hT)]
    hh_d = nc.declare_dram_parameter("hh", [G4, 128, 2 * PF], bf16, isOutput=False)
    cb_d = nc.declare_dram_parameter("cb", [128, CB_COLS], f32, isOutput=False)
    wih_d = nc.declare_dram_parameter("wih", [1, 4 * HD], bf16, isOutput=False)
    bsum_d = nc.declare_dram_parameter("bsum", [BL, 4 * HD], f32, isOutput=False)
    outd_d = nc.declare_dram_parameter("out_d", [BL, HD], f32, isOutput=True)
    outc_d = nc.declare_dram_parameter("out_c", [BL, HE], f32, isOutput=True)

    with TileContext(nc) as tc:
        with (
            tc.tile_pool(name="const", bufs=1) as constp,
            tc.tile_pool(name="hp", bufs=7) as hp,
            tc.tile_pool(name="re1p", bufs=3) as re1p,
            tc.tile_pool(name="smp", bufs=3) as smp,
            # tiles read by a DMA (gather/out): never recycle their slots, so
            # no engine op ever carries a WAR wait on a DMA semaphore
            tc.tile_pool(name="nrp", bufs=G4) as nrp,
            tc.tile_pool(name="tailp", bufs=1) as tailp,
            tc.tile_pool(name="udps", bufs=2, space="PSUM") as udps,
            tc.tile_pool(name="scps", bufs=1, space="PSUM") as scps,
            tc.tile_pool(name="trps", bufs=1, space="PSUM") as trps,
            tc.tile_pool(name="ctps", bufs=2, space="PSUM") as ctps,
        ):
            # ---- constants blob first (small; the matmuls need udt), then
            # the first h pairs, then the tail-only constants ----
            cb = constp.tile([128, CB_COLS], f32, tag="cb")
            nc.sync.dma_start(out=cb[:], in_=cb_d[:])
            prefetch = []
            for q in range(3):
                g0, qq0 = divmod(q, 2)
                t_ = hp.tile([128, PF], bf16, tag="h")
                nc.sync.dma_start(
                    out=t_[:], in_=hh_d[g0][:, qq0 * PF : (qq0 + 1) * PF]
                )
                prefetch.append(t_)
            udt_sb = [
                cb[:, CB_UDT0 : CB_UDT0 + 128].bitcast(bf16),
                cb[:, CB_UDT1 : CB_UDT1 + 128].bitcast(bf16),
            ]
            vd2_sb = cb[:, CB_VD2 : CB_VD2 + 1].bitcast(bf16)
            ident_sb = cb[:, CB_IDENT : CB_IDENT + 64].bitcast(bf16)
            wdb2_sb = cb[:, CB_WDB2 : CB_WDB2 + 2]
            wtr_sb = cb[:, CB_WTR : CB_WTR + HE]
            c2_sb = cb[:, CB_C2 : CB_C2 + G4]
            wih_sb = constp.tile([1, 4 * HD], bf16, tag="wih")
            nc.sync.dma_start(out=wih_sb[:], in_=wih_d[:])
            bsum_sb = constp.tile([BL, 4 * HD], f32, tag="bsum")
            nc.sync.dma_start(out=bsum_sb[:], in_=bsum_d[:])

            # ---- per-engine warmups ----
            # Every engine observes each constant-DMA semaphore via a cheap op
            # up front; later ops then need <=1 sync wait (the hardware
            # instruction structs encode only one wait command).
            warm = ctps.tile([128, T], f32, tag="ct")
            nc.tensor.matmul(
                warm[0:1, 0:1], vd2_sb[0:1, 0:1], vd2_sb[0:1, 0:1],
                start=True, stop=True,
            )
            nc.tensor.matmul(
                warm[0:1, 1:2], wih_sb[0:1, 0:1], wih_sb[0:1, 0:1],
                start=True, stop=True,
            )
            dscr = tailp.tile([1, 8], f32, tag="dscr")
            nc.vector.tensor_copy(dscr[0:1, 0:1], wtr_sb[0:1, 0:1])
            nc.vector.tensor_copy(dscr[0:1, 1:2], bsum_sb[0:1, 0:1])
            ascr = tailp.tile([1, 8], f32, tag="ascr")
            nc.scalar.copy(ascr[0:1, 0:1], wdb2_sb[0:1, 0:1])
            # chained: materialize the float-bias const AP on ACT, and force
            # the (single) tanh/exp table-set load before the main loop
            nc.scalar.activation(ascr[0:1, 1:2], ascr[0:1, 0:1], AF.Tanh, bias=0.0)

            ytacc = tailp.tile([128, G4], f32, tag="ytacc")

            def absorb(ps):
                # tiny const matmul into a freshly allocated PSUM tile: takes
                # over the slot-release wait so the first real matmul into the
                # tile carries only its own (single) cross-engine wait
                nc.tensor.matmul(
                    ps[0:1, 0:1], vd2_sb[0:1, 0:1], vd2_sb[0:1, 0:1],
                    start=True, stop=True,
                )

            # ---- per-group-of-4 pipeline ----
            # The beta-consuming PE work (transposes + c_t matmuls) of group g
            # is emitted AFTER group g+1's ud/sc matmuls: the PE queue is
            # strict FIFO, so placing the transpose right after the gather
            # would stall PE on the gather DMA latency even though the next
            # group's ud matmuls are ready.
            def flush(p):
                g, beta, rz, h_pair = p["g"], p["beta"], p["rz"], p["h_pair"]
                # transpose pexp -> pexpT columns [t=128 x 4tt, batch]
                btr = trps.tile([128, 32], bf16, tag="btr")
                absorb(btr[:, 0:2].bitcast(f32))
                for tt in range(4):
                    nc.tensor.transpose(
                        btr[:, tt * 4 : (tt + 1) * 4],
                        beta[:, tt * 128 : (tt + 1) * 128],
                        ident_sb[0:4, 0:4],
                    )
                betaT = smp.tile([128, 16], bf16, tag="betaT")
                nc.vector.tensor_copy(betaT[:], btr[:, 0:16])
                if DEBUG_STAGE <= 4:
                    nc.sync.dma_start(
                        out=outc_d[4 * g : 4 * g + 4, 0:16], in_=betaT[0:4, 0:16]
                    )
                    return
                # c_t for the 4 batches (unnormalized, then fused 1/Z)
                ct = ctps.tile([128, T], f32, tag="ct")
                absorb(ct)
                for jj in range(4):
                    qq, r = divmod(jj, 2)
                    for tt in range(4):
                        nc.tensor.matmul(
                            ct[32 * jj : 32 * jj + 32, 0:HE],
                            betaT[:, tt * 4 + jj : tt * 4 + jj + 1].broadcast_to(
                                [128, 32]
                            ),
                            h_pair[qq][:, (r * 4 + tt) * HE : (r * 4 + tt + 1) * HE],
                            start=(tt == 0),
                            stop=(tt == 3),
                            tile_position=(0, 32 * jj),
                        )
                ctstage = nrp.tile([128, HE], f32, tag="ctstage")
                nc.vector.tensor_scalar_mul(ctstage[:], ct[:, 0:HE], rz[:])
                # c_t output rows for this group (DMA un-strides the rows)
                nc.sync.dma_start(
                    out=outc_d[4 * g : 4 * g + 4, :],
                    in_=ctstage[:].rearrange("(j s) e -> j s e", s=32)[:, 0, :],
                )
                if DEBUG_STAGE <= 5:
                    return
                # y_tilde partial for this group: wt.c_t dot into this
                # group's ytacc column (tensor_tensor_reduce would fuse these
                # two but crashes the hardware — sim-only support)
                wscr = smp.tile([128, HE], f32, tag="wscr")
                nc.vector.tensor_mul(wscr[:], ctstage[:], wtr_sb[:])
                nc.vector.reduce_sum(
                    ytacc[:, g : g + 1], wscr[:], axis=mybir.AxisListType.X
                )

            pend = None
            for g in range(DEBUG_GROUPS):
                sc = scps.tile([128, T], f32, tag="sc")
                absorb(sc)
                h_pair = [None, None]
                for qq in range(2):  # two pairs of batches in this group
                    if 2 * g + qq < len(prefetch):
                        hh_sb = prefetch[2 * g + qq]
                    else:
                        hh_sb = hp.tile([128, PF], bf16, tag="h")
                        nc.sync.dma_start(
                            out=hh_sb[:], in_=hh_d[g][:, qq * PF : (qq + 1) * PF]
                        )
                    h_pair[qq] = hh_sb[:, 0:HBF]
                    hT_sb = hh_sb[:, HBF:PF]
                    # ud matmuls for both batches of the pair; ei-outer /
                    # r-inner keeps each stationary loaded for 2 matmuls
                    ud_eo = []
                    for eo in range(2):
                        ud = udps.tile([128, 2 * T], f32, tag="ud")
                        absorb(ud)
                        ud_eo.append(ud)
                        for ei in range(2):
                            for r in range(2):
                                nc.tensor.matmul(
                                    ud[:, r * T : (r + 1) * T],
                                    udt_sb[ei][:, eo * 128 : (eo + 1) * 128],
                                    hT_sb[:, (r * 2 + ei) * T : (r * 2 + ei + 1) * T],
                                    start=(ei == 0),
                                    stop=(ei == 1),
                                )
                    # tanh(+bias) over the pair -> re1 bf16 [128, (eo, r, t)]
                    re1 = re1p.tile([128, 2 * 2 * T], bf16, tag="re1")
                    for eo in range(2):
                        nc.scalar.activation(
                            re1[:, eo * 2 * T : (eo + 1) * 2 * T],
                            ud_eo[eo][:],
                            AF.Tanh,
                            bias=wdb2_sb[:, eo : eo + 1],
                        )
                    if DEBUG_STAGE <= 1:
                        continue
                    # scores for both batches -> psum rows {0,32,64,96}
                    # (r outer: each strip's accumulation group opens and
                    # closes before the next strip's begins — one pending
                    # group per PSUM bank)
                    for r in range(2):
                        for ei in range(2):
                            jj = 2 * qq + r
                            nc.tensor.matmul(
                                sc[32 * jj : 32 * jj + 32, :],
                                vd2_sb[:, ei : ei + 1].broadcast_to([128, 32]),
                                re1[:, (ei * 2 + r) * T : (ei * 2 + r + 1) * T],
                                start=(ei == 0),
                                stop=(ei == 1),
                                tile_position=(0, 32 * jj),
                            )

                if DEBUG_STAGE <= 1:
                    continue
                # ---- previous group's beta-consuming work goes on the PE
                # queue here, after this group's dense matmuls ----
                if pend is not None:
                    flush(pend)
                    pend = None
                # ---- softmax over this group of 4, directly on PSUM ----
                # scores are bounded (|s| <= sum|vd| ~ 10) so no max-shift is
                # needed; score rows are replicated across each 32-partition
                # strip, so every partition's zsum is its strip's batch value.
                pexp = nrp.tile([128, T], bf16, tag="pexp")
                zsum = smp.tile([128, 1], f32, tag="zsum")
                nc.scalar.activation(pexp[:], sc[:], AF.Exp, accum_out=zsum[:])
                rz = smp.tile([128, 1], f32, tag="rz")
                nc.vector.reciprocal(rz[:], zsum[:])
                # gather the 4 pexp rows into contiguous partitions (DMA moves
                # freely across partitions; engines cannot); SWDGE keeps this
                # latency-bound transfer off the bulk h ring and the busy ACT
                beta = smp.tile([4, T], bf16, tag="beta")
                # DVE dummy write absorbs the slot-release wait so the gather
                # DMA carries only its producer wait
                nc.vector.tensor_copy(beta[0:1, 0:1], dscr[0:1, 0:1])
                nc.gpsimd.dma_start(
                    out=beta[:],
                    in_=pexp[:].rearrange("(j s) t -> j s t", s=32)[:, 0, :],
                )
                pend = {"g": g, "beta": beta, "rz": rz, "h_pair": list(h_pair)}

            if pend is not None:
                flush(pend)
                pend = None

            # ---- tail: gates, LSTM cell (sigmoid via 0.5*(1+tanh(x/2))) ----
            if SKIP_TAIL or DEBUG_STAGE <= 5:
                pass
            else:
                yts = nrp.tile([128, G4], bf16, tag="yts")
                nc.vector.tensor_add(yts[:], ytacc[:], c2_sb[:])
                # gather strided y_tilde entries into one [1, BL] stationary
                # row; position p = j*8+g holds batch 4g+j (the scramble is
                # undone by the out_d DMA access pattern below)
                yt_row = tailp.tile([1, BL], bf16, tag="yt_row")
                nc.gpsimd.dma_start(
                    out=yt_row[:],
                    in_=yts[:].rearrange("(j s) g -> j s g", s=32)[:, 0, :],
                )
                gsb = tailp.tile([BL, 4 * HD], f32, tag="gsb")
                for half in range(2):
                    gps = ctps.tile([128, T], f32, tag="ct")
                    absorb(gps)
                    nc.tensor.matmul(
                        gps[0:BL, 0:512],
                        yt_row[:],
                        wih_sb[:, half * 512 : (half + 1) * 512],
                        start=True,
                        stop=True,
                    )
                    nc.vector.tensor_add(
                        gsb[:, half * 512 : (half + 1) * 512],
                        gps[0:BL, 0:512],
                        bsum_sb[:, half * 512 : (half + 1) * 512],
                    )
                # gates: i=[0:256], g=[512:768], o=[768:1024]  (f unused: c0=0)
                ti = tailp.tile([BL, HD], f32, tag="ti")
                nc.scalar.activation(ti[:], gsb[:, 0:256], AF.Tanh, scale=0.5)
                tg = tailp.tile([BL, HD], f32, tag="tg")
                nc.scalar.activation(tg[:], gsb[:, 512:768], AF.Tanh)
                to = tailp.tile([BL, HD], f32, tag="to")
                nc.scalar.activation(to[:], gsb[:, 768:1024], AF.Tanh, scale=0.5)
                # cn2 = (ti+1)*tg = 2*c_new ; tanh(0.5*cn2) = tanh(c_new)
                cn2 = tailp.tile([BL, HD], f32, tag="cn2")
                nc.vector.scalar_tensor_tensor(
                    out=cn2[:], in0=ti[:], scalar=1.0, in1=tg[:],
                    op0=ALU.add, op1=ALU.mult,
                )
                tcn = tailp.tile([BL, HD], f32, tag="tcn")
                nc.scalar.activation(tcn[:], cn2[:], AF.Tanh, scale=0.5)
                # dn2 = (to+1)*tcn = 2*d_new
                dn2 = tailp.tile([BL, HD], f32, tag="dn2")
                nc.vector.scalar_tensor_tensor(
                    out=dn2[:], in0=to[:], scalar=1.0, in1=tcn[:],
                    op0=ALU.add, op1=ALU.mult,
                )
                dnew = tailp.tile([BL, HD], f32, tag="dnew")
                nc.vector.tensor_scalar_mul(dnew[:], dn2[:], 0.5)

                # dnew row j*8+g is batch 4g+j
                nc.sync.dma_start(
                    out=outd_d[:].rearrange("(g j) e -> j g e", j=4), in_=dnew[:]
                )

    nc.compile()
    return nc


def _prep_in_maps(inputs):
    h = np.asarray(inputs["h_t_enc"], np.float32)
    y = np.asarray(inputs["y"], np.float32)
    Ud_w = np.asarray(inputs["Ud_w"], np.float32)
    Ud_b = np.asarray(inputs["Ud_b"], np.float32)
    Wd_b = np.asarray(inputs["Wd_b"], np.float32)
    vd_w = np.asarray(inputs["vd_w"], np.float32)
    wt_w = np.asarray(inputs["wt_w"], np.float32)
    wt_b = np.asarray(inputs["wt_b"], np.float32)
    W_ih = np.asarray(inputs["W_ih"], np.float32)
    b_ih = np.asarray(inputs["b_ih"], np.float32)
    b_hh = np.asarray(inputs["b_hh"], np.float32)

    from ml_dtypes import bfloat16

    udt = np.ascontiguousarray(Ud_w.T).astype(bfloat16)  # [256,256]
    vd2 = np.ascontiguousarray(vd_w[0].reshape(2, 128).T).astype(bfloat16)
    wdb2 = np.ascontiguousarray((Wd_b + Ud_b).reshape(2, 128).T)
    wtr = np.tile(wt_w[0][:HE][None, :], (128, 1)).astype(np.float32)
    wih = W_ih[:, 0][None, :].astype(bfloat16)
    bsum = np.tile((b_ih + b_hh)[None, :], (BL, 1)).astype(np.float32)
    ident = np.eye(128, dtype=bfloat16)

    def make_hh(hc):
        # h-region: per pair q, partition p: [rb(2), tt(4), e(256)] natural rows
        hp_ = np.ascontiguousarray(
            hc.reshape(BL // 2, 2, 4, 128, HE).transpose(0, 3, 1, 2, 4)
            .reshape(BL // 2, 128, 2048)
        ).astype(bfloat16)
        # hT-region: [rb(2), et(2), t(512)] transposed rows
        ht_ = np.ascontiguousarray(
            hc.transpose(0, 2, 1).reshape(BL // 2, 2, 2, 128, T)
            .transpose(0, 3, 1, 2, 4).reshape(BL // 2, 128, 2048)
        ).astype(bfloat16)
        pair = np.concatenate([hp_, ht_], axis=2)  # [BL//2, 128, PF]
        # group blob: both pairs of a group side by side per partition
        return np.ascontiguousarray(
            pair.reshape(G4, 2, 128, PF).transpose(0, 2, 1, 3)
            .reshape(G4, 128, 2 * PF)
        )

    def f32view(a):
        return np.ascontiguousarray(a).view(np.float32)

    in_maps = []
    for c in range(NCORES):
        sl = slice(c * BL, (c + 1) * BL)
        hc = h[sl]
        # per-batch constant part of y_tilde at strided layout [32*jj, g]
        c2v = wt_w[0, HE] * y[sl, 0, 0] + wt_b[0]  # [BL]
        c2 = np.zeros((128, G4), np.float32)
        for b in range(BL):
            c2[32 * (b % 4), b // 4] = c2v[b]
        cbl = np.concatenate(
            [
                f32view(udt[0:128]),
                f32view(udt[128:256]),
                f32view(vd2),
                f32view(ident),
                wdb2,
                wtr,
                c2,
            ],
            axis=1,
        )
        assert cbl.shape == (128, CB_COLS), cbl.shape
        in_maps.append(
            {
                "hh": make_hh(hc),
                "cb": np.ascontiguousarray(cbl),
                "wih": wih,
                "bsum": bsum,
            }
        )
    return in_maps


def kernel(**inputs):
    from concourse.bass_utils import run_bass_kernel_spmd

    key = 0
    if key not in _cache:
        _cache[key] = _build_nc()
    nc = _cache[key]

    in_maps = _prep_in_maps(inputs)
    res = run_bass_kernel_spmd(nc, in_maps, list(range(NCORES)))
    kernel.last_results = res

    d_new = np.concatenate([np.asarray(r["out_d"]) for r in res.results], axis=0)
    c_t = np.concatenate([np.asarray(r["out_c"]) for r in res.results], axis=0)
    return np.stack([d_new.astype(np.float32), c_t.astype(np.float32)], axis=0)


kernel.last_results = None
